# revision 1
# baseline (speedup 1.0000x reference)
"""Cross-WindowAttention Trainium2 kernel.

Full inputs -> shard batch dim over 8 NeuronCores -> bass/Tile kernel per core
-> gather. Host-side numpy does layout prep (transposes to feature-major,
bf16 conversion, combined rpb+mask bias table); the Bass kernel does all
matmul/softmax compute.

Per-core pipeline (shard = 256 windows of 64 tokens, 16384 rows):
 - qkv projections on PE in bf16, contraction over concat(x,y) for k/v.
   q,k produced feature-major [feat, rows]; v row-major per window [64, 512].
 - attention per (head-pair, 8-window chunk) in one [128, 512] PSUM bank:
   bias added via identity-matmul accumulation, exp on ScalarE (scale folded),
   softmax denominator via VectorE free-axis reduce + reciprocal + bcast mul,
   PE-transpose of normalized weights, PV matmul with v stationary.
 - output projection with attention-output tiles stationary -> row-major
   result, biases via ones-row matmul, contiguous DMA out.

The chunk loop is software-pipelined by one chunk: the small attention/proj
matmul groups of chunk c-1 are emitted interleaved between the large qkv
matmul groups of chunk c, keeping the PE array duty cycle high enough that
the HAM activity monitor does not clock-gate it to half speed.
"""
import numpy as np
import ml_dtypes

import concourse.bacc as bacc
import concourse.mybir as mybir
from concourse.tile import TileContext
from concourse.bass_utils import run_bass_kernel_spmd

F32 = mybir.dt.float32
BF16 = mybir.dt.bfloat16
BF = ml_dtypes.bfloat16

N_CORES = 8
B_FULL = 2048
N = 64                      # window size (tokens per window)
C = 512                     # channels
H = 16                      # heads
HD = 32                     # head dim
CX = 512                    # x feature dim
CY = 1000                   # y feature dim
CYP = 1024                  # y feature dim padded to multiple of 128
SCALE = HD ** -0.5

B_SHARD = B_FULL // N_CORES             # 256 windows per core
WIN_PER_CHUNK = 8
ROWS_PER_CHUNK = WIN_PER_CHUNK * N      # 512
N_CHUNKS = B_SHARD // WIN_PER_CHUNK     # 32

KT_X = CX // 128            # 4 contraction tiles from x
KT_Y = CYP // 128           # 8 contraction tiles from y (padded)
FT_Q = C // 128             # 4 feature tiles per projection output


def build_nc(n_chunks=N_CHUNKS):
    rows = n_chunks * ROWS_PER_CHUNK
    nc = bacc.Bacc("TRN2", target_bir_lowering=False)

    xt = nc.dram_tensor("xt", [CX, rows], BF16, kind="ExternalInput")
    yt = nc.dram_tensor("yt", [CYP, rows], BF16, kind="ExternalInput")
    w1 = nc.dram_tensor("w1", [CX, 3 * C], BF16, kind="ExternalInput")
    w2 = nc.dram_tensor("w2", [CYP, 3 * C], BF16, kind="ExternalInput")
    wp = nc.dram_tensor("wp", [4, 128, C], BF16, kind="ExternalInput")  # quad-permuted rows
    cb = nc.dram_tensor("cb", [8, 8, 128, 512], BF16, kind="ExternalInput")
    bq = nc.dram_tensor("bq", [128, FT_Q], F32, kind="ExternalInput")
    bp = nc.dram_tensor("bp", [128, C], F32, kind="ExternalInput")
    ident = nc.dram_tensor("ident", [128, 128], BF16, kind="ExternalInput")
    out = nc.dram_tensor("out", [rows, C], F32, kind="ExternalOutput")

    with TileContext(nc) as tc:
        with tc.tile_pool(name="const", bufs=1) as constp, \
             tc.tile_pool(name="wpool", bufs=1) as wpool, \
             tc.tile_pool(name="stream", bufs=2) as stream, \
             tc.tile_pool(name="acts", bufs=2) as acts, \
             tc.tile_pool(name="small", bufs=3) as small, \
             tc.tile_pool(name="pbig", bufs=2, space="PSUM") as pbig, \
             tc.tile_pool(name="pattn", bufs=2, space="PSUM") as pattn, \
             tc.tile_pool(name="ptnk", bufs=2, space="PSUM") as ptnk, \
             tc.tile_pool(name="pot", bufs=2, space="PSUM") as pot:

            # ---- resident constants / weights
            w1_sb = wpool.tile([128, KT_X, 3 * C], BF16)
            nc.sync.dma_start(out=w1_sb, in_=w1.rearrange("(a p) f -> p a f", p=128))
            w2_sb = wpool.tile([128, KT_Y, 3 * C], BF16)
            nc.sync.dma_start(out=w2_sb, in_=w2.rearrange("(a p) f -> p a f", p=128))
            wp_sb = wpool.tile([128, 4, C], BF16)
            nc.sync.dma_start(out=wp_sb, in_=wp.rearrange("a p f -> p a f"))
            bq_sb = constp.tile([128, FT_Q], F32)
            nc.sync.dma_start(out=bq_sb, in_=bq[:, :])
            bp_sb = constp.tile([128, C], F32)
            nc.sync.dma_start(out=bp_sb, in_=bp[:, :])
            id_sb = constp.tile([128, 128], BF16)
            nc.sync.dma_start(out=id_sb, in_=ident[:, :])

            xt_r = xt.rearrange("(a p) r -> p a r", p=128)
            yt_r = yt.rearrange("(a p) r -> p a r", p=128)

            st = {}  # per-chunk live tiles

            def emit_dma(c):
                r0 = c * ROWS_PER_CHUNK
                s = {}
                s["xt"] = stream.tile([128, KT_X, ROWS_PER_CHUNK], BF16, tag="xt", name="xt")
                nc.sync.dma_start(out=s["xt"], in_=xt_r[:, :, r0:r0 + ROWS_PER_CHUNK])
                s["yt"] = stream.tile([128, KT_Y, ROWS_PER_CHUNK], BF16, tag="yt", name="yt")
                nc.sync.dma_start(out=s["yt"], in_=yt_r[:, :, r0:r0 + ROWS_PER_CHUNK])
                s["cb"] = stream.tile([128, 8, 512], BF16, tag="cb", name="cbt")
                nc.sync.dma_start(out=s["cb"],
                                  in_=cb[c % 8].rearrange("hp p f -> p hp f"))
                s["q"] = acts.tile([128, FT_Q, ROWS_PER_CHUNK], BF16, tag="q", name="qsb")
                s["k"] = acts.tile([128, FT_Q, ROWS_PER_CHUNK], BF16, tag="k", name="ksb")
                s["v"] = acts.tile([64, WIN_PER_CHUNK, C], BF16, tag="v", name="vsb")
                s["ot"] = acts.tile([128, 4 * ROWS_PER_CHUNK], BF16, tag="ot", name="otsb")
                st[c] = s

            def emit_qkv_group(c, g):
                s = st[c]
                if g < FT_Q:                      # q projection, feature tile g
                    ft = g
                    bank = pbig.tile([128, ROWS_PER_CHUNK], F32, tag="pq")
                    for kt in range(KT_X):
                        nc.tensor.matmul(
                            bank[:, :],
                            w1_sb[:, kt, 128 * ft:128 * (ft + 1)],
                            s["xt"][:, kt, :],
                            start=(kt == 0), stop=(kt == KT_X - 1))
                    nc.scalar.activation(
                        s["q"][:, ft, :], bank[:, :],
                        mybir.ActivationFunctionType.Identity,
                        bias=bq_sb[:, ft:ft + 1])
                elif g < 2 * FT_Q:                # k projection, feature tile g-4
                    ft = g - FT_Q
                    bank = pbig.tile([128, ROWS_PER_CHUNK], F32, tag="pq")
                    for kt in range(KT_X):
                        nc.tensor.matmul(
                            bank[:, :],
                            w1_sb[:, kt, C + 128 * ft:C + 128 * (ft + 1)],
                            s["xt"][:, kt, :],
                            start=(kt == 0), stop=False)
                    for kt in range(KT_Y):
                        nc.tensor.matmul(
                            bank[:, :],
                            w2_sb[:, kt, C + 128 * ft:C + 128 * (ft + 1)],
                            s["yt"][:, kt, :],
                            start=False, stop=(kt == KT_Y - 1))
                    nc.scalar.copy(s["k"][:, ft, :], bank[:, :])
                else:                             # v projection, row tile g-8
                    rt = g - 2 * FT_Q
                    bank = pbig.tile([128, C], F32, tag="pq")
                    for kt in range(KT_X):
                        nc.tensor.matmul(
                            bank[:, :],
                            s["xt"][:, kt, 128 * rt:128 * (rt + 1)],
                            w1_sb[:, kt, 2 * C:3 * C],
                            start=(kt == 0), stop=False)
                    for kt in range(KT_Y):
                        nc.tensor.matmul(
                            bank[:, :],
                            s["yt"][:, kt, 128 * rt:128 * (rt + 1)],
                            w2_sb[:, kt, 2 * C:3 * C],
                            start=False, stop=(kt == KT_Y - 1))
                    nc.scalar.copy(s["v"][:, 2 * rt, :], bank[0:64, :])
                    nc.vector.tensor_copy(s["v"][:, 2 * rt + 1, :], bank[64:128, :])

            def emit_attn_group(c, hp):
                s = st[c]
                bank = pattn.tile([128, 512], F32, tag="pattn")
                for sw in range(WIN_PER_CHUNK):
                    for hh in range(2):
                        h = 2 * hp + hh
                        pq = 32 * (h % 4)
                        ft = h // 4
                        nc.tensor.matmul(
                            bank[64 * hh:64 * (hh + 1), 64 * sw:64 * (sw + 1)],
                            s["q"][pq:pq + 32, ft, 64 * sw:64 * (sw + 1)],
                            s["k"][pq:pq + 32, ft, 64 * sw:64 * (sw + 1)],
                            start=True, stop=True, skip_group_check=True,
                            tile_position=(pq, 64 * hh))
                # combined rpb+mask bias (pre-divided by SCALE) added on DVE
                nc.vector.tensor_tensor(out=bank[:, :], in0=bank[:, :],
                                        in1=s["cb"][:, hp, :],
                                        op=mybir.AluOpType.add)
                expa = small.tile([128, 8, 64], BF16, tag="expa")
                nc.scalar.activation(
                    expa.rearrange("p s m -> p (s m)"), bank[:, :],
                    mybir.ActivationFunctionType.Exp, scale=SCALE)
                den = small.tile([128, 8], F32, tag="den")
                nc.vector.tensor_reduce(
                    den[:, :], expa[:, :, :],
                    axis=mybir.AxisListType.X, op=mybir.AluOpType.add)
                rden = small.tile([128, 8], F32, tag="rden")
                nc.vector.reciprocal(rden[:, :], den[:, :])
                norma = small.tile([128, 8, 64], BF16, tag="norma")
                nc.vector.tensor_tensor(
                    out=norma[:, :, :], in0=expa[:, :, :],
                    in1=rden.unsqueeze(-1).broadcast_to([128, 8, 64]),
                    op=mybir.AluOpType.mult)
                # transpose normalized weights: [(2h,n), m] -> [m, (2h,n)]
                tnk_sb = small.tile([64, 8, 128], BF16, tag="tnk")
                for half in range(2):
                    tbank = ptnk.tile([64, 512], BF16, tag="ptnk")
                    for j in range(4):
                        sw = 4 * half + j
                        nc.tensor.transpose(
                            tbank[:, 128 * j:128 * (j + 1)],
                            norma[:, sw, :], id_sb[:, :])
                    dst = tnk_sb[:, 4 * half:4 * half + 4, :] \
                        .rearrange("p s f -> p (s f)")
                    if half == 0:
                        nc.scalar.copy(dst, tbank[:, :])
                    else:
                        nc.vector.tensor_copy(dst, tbank[:, :])
                # PV: v stationary, transposed attn moving
                obank = pot.tile([64, 512], F32, tag="pot")
                for sw in range(WIN_PER_CHUNK):
                    for hh in range(2):
                        h = 2 * hp + hh
                        nc.tensor.matmul(
                            obank[32 * hh:32 * (hh + 1), 64 * sw:64 * (sw + 1)],
                            s["v"][:, sw, HD * h:HD * (h + 1)],
                            tnk_sb[:, sw, 64 * hh:64 * (hh + 1)],
                            start=True, stop=True)
                # stage to SBUF: partition 32*(h%4)+d, free (t, q=h//4, w, m)
                nc.scalar.copy(
                    s["ot"][64 * (hp % 2):64 * (hp % 2) + 64, :]
                    .rearrange("p (t q w m) -> p t q w m", t=4, q=4, w=2)
                    [:, :, hp // 2, :, :],
                    obank.rearrange("p (t w m) -> p t w m", t=4, w=2))

            def emit_proj_group(c, rt):
                s = st[c]
                r0 = c * ROWS_PER_CHUNK
                bank = pbig.tile([128, C], F32, tag="pq")
                for quad in range(4):
                    nc.tensor.matmul(
                        bank[:, :],
                        s["ot"].rearrange("p (t q f) -> p t q f", t=4, q=4)
                        [:, rt, quad, :],
                        wp_sb[:, quad, :],
                        start=(quad == 0), stop=(quad == 3))
                out_f32 = small.tile([128, C], F32, tag="outf")
                nc.vector.tensor_tensor(out=out_f32[:, :], in0=bank[:, :],
                                        in1=bp_sb[:, :], op=mybir.AluOpType.add)
                nc.sync.dma_start(
                    out=out[r0 + 128 * rt:r0 + 128 * (rt + 1), :],
                    in_=out_f32[:, :])

            # software pipeline: big qkv groups of chunk c interleaved with
            # small attention/proj groups of chunk c-1
            for c in range(n_chunks + 1):
                if c < n_chunks:
                    emit_dma(c)
                big = [("qkv", c, g) for g in range(12)] if c < n_chunks else []
                smalls = ([("attn", c - 1, hp) for hp in range(8)]
                          + [("proj", c - 1, rt) for rt in range(4)]) if c > 0 else []
                order = []
                for i in range(max(len(big), len(smalls))):
                    if i < len(big):
                        order.append(big[i])
                    if i < len(smalls):
                        order.append(smalls[i])
                for kind, cc, idx in order:
                    if kind == "qkv":
                        emit_qkv_group(cc, idx)
                    elif kind == "attn":
                        emit_attn_group(cc, idx)
                    else:
                        emit_proj_group(cc, idx)
                if c > 0:
                    del st[c - 1]
    nc.compile()
    return nc


_NC_CACHE = {}


def _get_nc(n_chunks):
    if n_chunks not in _NC_CACHE:
        _NC_CACHE[n_chunks] = build_nc(n_chunks)
    return _NC_CACHE[n_chunks]


def prep_shared(w_qkv1, b_qkv1, w_qkv2, b_qkv2, bias_table, rel_index, w_proj,
                b_proj, mask):
    """Host-side prep of weights/bias tables shared by all cores."""
    w1 = w_qkv1.astype(BF)
    w2 = np.zeros((CYP, 3 * C), np.float32)
    w2[:CY] = w_qkv2
    # k/v biases ride an all-ones row in the padded region of yT
    w2[CY, C:2 * C] = b_qkv1[C:2 * C] + b_qkv2[C:2 * C]
    w2[CY, 2 * C:] = b_qkv1[2 * C:] + b_qkv2[2 * C:]
    w2 = w2.astype(BF)
    # quad-permuted rows: wp[Q, p, :] = w_proj[32*(4Q + p//32) + p%32, :]
    wp = np.empty((4, 128, C), np.float32)
    for q in range(4):
        for g in range(4):
            wp[q, 32 * g:32 * (g + 1), :] = \
                w_proj[32 * (4 * q + g):32 * (4 * q + g) + 32, :]
    wp = wp.astype(BF)

    bq = b_qkv1[0:C].reshape(FT_Q, 128).T.astype(np.float32).copy()
    bp = np.broadcast_to(b_proj.astype(np.float32), (128, C)).copy()

    rpb = bias_table[rel_index.reshape(-1)].reshape(N, N, H).transpose(2, 0, 1)
    cbt = (rpb[None] + mask[:, None]) / SCALE          # [w, h, n, m]
    cb6 = cbt.reshape(8, 8, 8, 2, N, N)                # [c8, s, hp, hh, n, m]
    cbd = np.ascontiguousarray(cb6.transpose(0, 2, 3, 4, 1, 5)) \
        .reshape(8, 8, 128, 512).astype(BF)

    ident = np.eye(128, dtype=BF)
    return dict(w1=w1, w2=w2, wp=wp, bq=bq, bp=bp, cb=cbd, ident=ident)


def prep_core_inputs(x, y, shared, n_cores=N_CORES):
    """Split x, y along batch, transpose to feature-major, bf16."""
    B_, n, _ = x.shape
    rows = (B_ // n_cores) * n
    in_maps = []
    for i in range(n_cores):
        lo = i * (B_ // n_cores)
        hi = lo + B_ // n_cores
        xs = x[lo:hi].reshape(rows, CX)
        ys = y[lo:hi].reshape(rows, CY)
        xtb = np.ascontiguousarray(xs.T).astype(BF)
        ytb = np.zeros((CYP, rows), BF)
        ytb[:CY] = np.ascontiguousarray(ys.T).astype(BF)
        ytb[CY] = 1.0
        in_maps.append(dict(xt=xtb, yt=ytb, **shared))
    return in_maps


def kernel(x, y, mask, w_qkv1, b_qkv1, w_qkv2, b_qkv2, bias_table, rel_index,
           w_proj, b_proj, _n_cores=N_CORES, _trace=False):
    B_, n, _ = x.shape
    n_chunks = (B_ // _n_cores) // WIN_PER_CHUNK
    shared = prep_shared(np.asarray(w_qkv1), np.asarray(b_qkv1),
                         np.asarray(w_qkv2), np.asarray(b_qkv2),
                         np.asarray(bias_table), np.asarray(rel_index),
                         np.asarray(w_proj), np.asarray(b_proj),
                         np.asarray(mask))
    in_maps = prep_core_inputs(np.asarray(x), np.asarray(y), shared, _n_cores)
    nc = _get_nc(n_chunks)
    res = run_bass_kernel_spmd(nc, in_maps, core_ids=list(range(_n_cores)),
                               trace=_trace)
    outs = [res.results[i]["out"].reshape(B_ // _n_cores, n, C)
            for i in range(_n_cores)]
    full = np.concatenate(outs, axis=0)
    kernel.last_results = res
    return full

